# revision 20
# baseline (speedup 1.0000x reference)
"""CapsuleLayer (dynamic routing, 3 iterations) Trainium2 Bass kernel.

Problem: inputs [64, 2048, 16] f32, W [1, 2048, 32, 16, 16] f32
  inputs_hat[b,n,o,d] = sum_i W[n,o,d,i] * inputs[b,n,i]
  3 routing iterations (softmax over o); only the last s/squash matters, and the
  b-update never uses `outputs`, so the whole computation collapses to:
    ihsum[b,n,o] = sum_d ih[b,n,o,d]            (= x . Wsum)
    e1 = exp(ihsum/32); c1 = e1 / sum_o e1
    b2 = ihsum*(1/32 + c1); e2 = exp(b2); r2 = 1/sum_o e2   (c2 = e2*r2)
    s[b,o,d] = sum_n c2[b,n,o] * ih[b,n,o,d]
    out = squash(s)

Sharding: Ni (2048) split 8 ways (256 capsules per core). Routing is local per
(b, n); each core produces a partial s [64, 32, 16] which the host sums and
squashes (tiny: 32K elements).

Per-core device pipeline (all matmuls bf16, PSUM f32):
  pass 1: ihsum via per-capsule matmuls, x stationary (4 capsules col-tiled,
          batch split in halves of 32), batched 16 units per PSUM bank.
  routing: ACT exp (PSUM src) + DVE reduces/recip + GPSIMD tensor ops,
          producing e2 (bf16) and r2 (f32) per block.
  pass 2: per unit (4 capsules x 32 batch): ih into PSUM [128, 512]; then a
          fused DVE scalar_tensor_tensor (tmp = (psum_ih * r2) * e2-broadcast)
          or an ACT(scale=r2 copy) + GPSIMD multiply; then a PE contraction
          with a fixed 0/1 selector E [(4n,32b) -> 32b] accumulating s in PSUM.
"""

import os
import sys

import numpy as np
import ml_dtypes

sys.path.insert(0, "/opt/trn_rl_repo")
sys.path.insert(0, "/opt/pypackages")

import concourse.bass as bass
import concourse.mybir as mybir
import concourse.tile as tile
from concourse import bacc
from concourse.bass_utils import run_bass_kernel_spmd

BF16 = mybir.dt.bfloat16
F32 = mybir.dt.float32
AF = mybir.ActivationFunctionType
OP = mybir.AluOpType

B, NI, DI, NO, DO = 64, 2048, 16, 32, 16
NCORES = 8
NL = NI // NCORES            # 256 capsules per core
OD = NO * DO                 # 512
NQ = NL // 4                 # 64 quads (4 capsules each)
NUNITS = NQ * 2              # 128 units: (quad, batch-half)
UNITS_PER_BLOCK = 16         # routing block: 16 units -> psum [128, 512]
NBLOCKS = NUNITS // UNITS_PER_BLOCK   # 8
WCHUNK_Q = 8                 # quads per W dma chunk (32 capsules)
EPS = 1e-7
# units routed through DVE scalar_tensor_tensor vs ACT+GPSIMD, per 16 units
DVE_ROUTE = tuple(range(10))  # 10/16 on DVE, 6/16 on ACT+GPSIMD


def _build_program():
    nc = bacc.Bacc("TRN2", target_bir_lowering=False, debug=False)

    x_d = nc.dram_tensor("x", [64, NQ, 2, 128], BF16, kind="ExternalInput").ap()
    w_d = nc.dram_tensor("w", [NQ, 64, OD], BF16, kind="ExternalInput").ap()
    ws_d = nc.dram_tensor("ws", [64, NQ, NO], BF16, kind="ExternalInput").ap()
    es_d = nc.dram_tensor("esel", [128, 32], BF16, kind="ExternalInput").ap()
    s_d = nc.dram_tensor("s_out", [64, OD], F32, kind="ExternalOutput").ap()

    with tile.TileContext(nc) as tc:
        _emit(tc, x_d, w_d, ws_d, es_d, s_d)
    nc.compile()
    return nc


def _emit(tc, x_d, w_d, ws_d, es_d, s_d):
    nc = tc.nc
    from contextlib import ExitStack

    ctx = ExitStack()
    const = ctx.enter_context(tc.tile_pool(name="const", bufs=1))
    wpool = ctx.enter_context(tc.tile_pool(name="w", bufs=2))
    rpool = ctx.enter_context(tc.tile_pool(name="routing", bufs=3))
    spool = ctx.enter_context(tc.tile_pool(name="small", bufs=3))
    e2pool = ctx.enter_context(tc.tile_pool(name="e2", bufs=3))
    r2pool = ctx.enter_context(tc.tile_pool(name="r2", bufs=3))
    tmppool = ctx.enter_context(tc.tile_pool(name="tmp", bufs=6))
    outpool = ctx.enter_context(tc.tile_pool(name="out", bufs=1))
    ps1pool = ctx.enter_context(tc.tile_pool(name="ps1", bufs=3, space="PSUM"))
    psihpool = ctx.enter_context(tc.tile_pool(name="psih", bufs=4, space="PSUM"))
    psspool = ctx.enter_context(tc.tile_pool(name="pss", bufs=1, space="PSUM"))

    # resident inputs
    # x: block-diagonal stationary per (quad, half): [64 = (4n,16i), 128 = (4n,32b)]
    x_sb = const.tile([64, NQ, 2, 128], BF16)
    nc.sync.dma_start(x_sb[:], x_d[:])
    ws_sb = const.tile([64, NQ, NO], BF16)
    nc.sync.dma_start(ws_sb[:], ws_d[:])
    es_sb = const.tile([128, 32], BF16)
    nc.sync.dma_start(es_sb[:], es_d[:])

    # s accumulator psum, lives across the whole pass 2
    ps_s = psspool.tile([128, OD], F32)
    s_written = [False] * 4  # per col-group start flag

    e2_blocks = [None] * NBLOCKS
    r2_blocks = [None] * NBLOCKS
    e1_blocks = [None] * NBLOCKS

    # ---------------- pass 1: ihsum + routing ----------------
    for blk in range(NBLOCKS):
        ps1 = ps1pool.tile([128, UNITS_PER_BLOCK * NO], F32)
        for j in range(UNITS_PER_BLOCK):
            u = blk * UNITS_PER_BLOCK + j
            q, h = u // 2, u % 2
            nc.tensor.matmul(
                ps1[:, 32 * j:32 * (j + 1)],
                lhsT=x_sb[:, q, h, :],
                rhs=ws_sb[:, q, :],
                start=True, stop=True,
            )
        # routing on this block  (free dim = (unit j, o))
        # single PSUM reader so the bank-reuse WAR resolves to one semaphore
        ihs = rpool.tile([128, UNITS_PER_BLOCK, NO], F32, tag="ihs")
        nc.scalar.copy(ihs[:], ps1.rearrange("p (j o) -> p j o", o=NO))
        e1 = rpool.tile([128, UNITS_PER_BLOCK, NO], BF16, tag="e1")
        nc.scalar.activation(e1[:], ihs[:], AF.Exp, scale=1.0 / 32.0)
        z1 = spool.tile([128, UNITS_PER_BLOCK], F32, tag="z1")
        nc.vector.tensor_reduce(z1[:], e1[:], axis=mybir.AxisListType.X, op=OP.add)
        r1 = spool.tile([128, UNITS_PER_BLOCK], F32, tag="r1")
        nc.vector.reciprocal(r1[:], z1[:])
        # u1 = e1 * r1 (broadcast over o), then += 1/32, then t2 = ihs * u1
        u1 = rpool.tile([128, UNITS_PER_BLOCK, NO], BF16, tag="u1")
        nc.gpsimd.tensor_tensor(
            u1[:], e1[:], r1[:, :, None].to_broadcast((128, UNITS_PER_BLOCK, NO)),
            op=OP.mult,
        )
        nc.gpsimd.tensor_scalar_add(u1[:], u1[:], 1.0 / 32.0)
        t2 = rpool.tile([128, UNITS_PER_BLOCK, NO], BF16, tag="t2")
        nc.gpsimd.tensor_tensor(t2[:], ihs[:], u1[:], op=OP.mult)
        e2 = e2pool.tile([128, UNITS_PER_BLOCK, NO], BF16, tag="e2")
        nc.scalar.activation(e2[:], t2[:], AF.Exp)
        z2 = spool.tile([128, UNITS_PER_BLOCK], F32, tag="z2")
        nc.vector.tensor_reduce(z2[:], e2[:], axis=mybir.AxisListType.X, op=OP.add)
        r2 = r2pool.tile([128, UNITS_PER_BLOCK], F32, tag="r2")
        nc.vector.reciprocal(r2[:], z2[:])
        e2_blocks[blk] = e2
        r2_blocks[blk] = r2
        e1_blocks[blk] = e1

    # ---------------- pass 2: ih, weighting, contraction ----------------
    w_tile = None
    tmp_tiles = [None] * NUNITS
    for u in range(NUNITS):
        q, h = u // 2, u % 2
        blk, j = u // UNITS_PER_BLOCK, u % UNITS_PER_BLOCK
        if h == 0 and q % WCHUNK_Q == 0:
            q0 = q
            w_tile = wpool.tile([64, WCHUNK_Q, OD], BF16, tag="wt")
            nc.sync.dma_start(
                w_tile[:], w_d[q0:q0 + WCHUNK_Q].rearrange("q p f -> p q f")
            )
        ps_ih = psihpool.tile([128, OD], F32)
        nc.tensor.matmul(
            ps_ih[:],
            lhsT=x_sb[:, q, h, :],
            rhs=w_tile[:, q - q0, :],
            start=True, stop=True,
        )
        e2 = e2_blocks[blk]
        r2 = r2_blocks[blk]
        e2_b = e2[:, j, :, None].to_broadcast((128, NO, DO))
        r2_s = r2[:, j:j + 1]
        tmp = tmppool.tile([128, NO, DO], BF16, tag="tmp")
        if u % 16 in DVE_ROUTE:
            nc.vector.scalar_tensor_tensor(
                tmp[:], ps_ih.rearrange("p (o d) -> p o d", d=DO), r2_s, e2_b,
                op0=OP.mult, op1=OP.mult,
            )
        else:
            ihr = tmppool.tile([128, NO, DO], BF16, tag="ihr")
            nc.scalar.activation(
                ihr[:], ps_ih.rearrange("p (o d) -> p o d", d=DO), AF.Copy,
                scale=r2_s,
            )
            nc.gpsimd.tensor_tensor(tmp[:], ihr[:], e2_b, op=OP.mult)
        tmp_tiles[u] = tmp
        gsel = 2 * h + (q & 1)
        nc.tensor.matmul(
            ps_s[32 * gsel:32 * (gsel + 1), :],
            lhsT=es_sb[:],
            rhs=tmp.rearrange("p o d -> p (o d)"),
            start=not s_written[gsel], stop=(u >= NUNITS - 4),
            tile_position=(0, 32 * gsel),
        )
        s_written[gsel] = True

    # ---------------- epilogue: fold 4 groups -> s [64, 512] ----------------
    # (tensor_tensor may read at most one PSUM operand: stage one side via ACT)
    s_tmp = outpool.tile([64, OD], F32, tag="s_tmp")
    nc.scalar.copy(s_tmp[0:32, :], ps_s[0:32, :])
    nc.scalar.copy(s_tmp[32:64, :], ps_s[64:96, :])
    s_sb = outpool.tile([64, OD], F32, tag="s_sb")
    nc.vector.tensor_tensor(s_sb[0:32, :], s_tmp[0:32, :], ps_s[32:64, :], op=OP.add)
    nc.vector.tensor_tensor(s_sb[32:64, :], s_tmp[32:64, :], ps_s[96:128, :], op=OP.add)
    nc.sync.dma_start(s_d[:], s_sb[:])
    ctx.close()


_NC_CACHE = None


def _get_program():
    global _NC_CACHE
    if _NC_CACHE is None:
        _NC_CACHE = _build_program()
    return _NC_CACHE


def kernel(inputs: np.ndarray, W: np.ndarray) -> np.ndarray:
    inputs = np.asarray(inputs, dtype=np.float32)
    W = np.asarray(W, dtype=np.float32)

    bf16 = ml_dtypes.bfloat16
    NQT = NI // 4  # quads over the full Ni
    # x block-diagonal stationaries: [NQT, 2, 4, 16, 4, 32] with blocks on the
    # (g, g) diagonal; block (q, h, g) = inputs[32h:32h+32, 4q+g, :].T
    xt = inputs.transpose(1, 2, 0)            # [Ni, Di, B]
    src = xt.reshape(NQT, 4, DI, 2, 32)       # [q, g, i, h, b]
    x4 = np.zeros((NQT, 2, 4, DI, 4, 32), dtype=np.float32)
    for g in range(4):
        x4[:, :, g, :, g, :] = src[:, g].transpose(0, 2, 1, 3)  # [q, h, i, b]
    x4 = x4.reshape(NQT, 2, 64, 128).transpose(2, 0, 1, 3)      # [64, q, h, 128]
    x4 = np.ascontiguousarray(x4).astype(bf16)
    # W: [1, Ni, No, Do, Di] -> [q, (g,i)=64, No*Do]
    w4 = np.ascontiguousarray(
        W[0].transpose(0, 3, 1, 2).reshape(NQT, 4 * DI, OD)).astype(bf16)
    # Wsum over Do: [Ni, No, Di] -> [(g,i)=64, q, No]
    ws4 = W[0].sum(axis=2).transpose(0, 2, 1).reshape(NQT, 4 * DI, NO)
    ws4 = np.ascontiguousarray(ws4.transpose(1, 0, 2)).astype(bf16)  # [64, q, No]
    esel = np.tile(np.eye(32, dtype=np.float32), (4, 1)).astype(bf16)

    nc = _get_program()
    in_maps = []
    for c in range(NCORES):
        sl = slice(c * NQ, (c + 1) * NQ)
        in_maps.append({
            "x": np.ascontiguousarray(x4[:, sl]),
            "w": np.ascontiguousarray(w4[sl]),
            "ws": np.ascontiguousarray(ws4[:, sl]),
            "esel": esel,
        })
    res = run_bass_kernel_spmd(nc, in_maps, core_ids=list(range(NCORES)))
    s = np.zeros((64, OD), dtype=np.float32)
    for r in res.results:
        s += np.asarray(r["s_out"], dtype=np.float32)
    s = s.reshape(B, NO, DO)
    s2 = np.sum(np.square(s), axis=-1, keepdims=True)
    scale = s2 / (1.0 + s2) / np.sqrt(s2 + EPS)
    return (scale * s).astype(np.float32)
